# revision 2
# baseline (speedup 1.0000x reference)
"""Trainium2 Bass kernel for the DigitCaps layer (capsule routing).

Math note: in the reference, the routing logits `raw` are never updated
inside the loop (the original model's bug, replicated faithfully), so
c = softmax(zeros) = 1/10 on every iteration and the whole module
collapses to

    v = squash(0.1 * einsum('ijkd,bid->bjk', W[0], x))

i.e. a single matmul  s[b, jk] = x_flat[b, :] @ Wmat[:, jk]  with
x_flat [B, 9216], Wmat [9216, 160] (k-index = (i, d), col = (j, k)),
followed by a per-(b, j) squash over the 16-dim capsule vectors.

Sharding (8 cores, no collectives): batch 4-way x capsule-half 2-way.
Each core computes s_c [64, 80] = xT_c^T @ W_c as 72 PSUM-accumulated
K=128 matmuls, then squashes on-chip. Per-core HBM traffic is
2.36 MB (x quarter) + 2.95 MB (W half) ~= 5.3 MB, the minimum for a
collective-free layout.

Layout trick: the contraction order over k = (i, d) is arbitrary as long
as both operands agree, so host-side we lay out both operands as
[128 partitions, 72 ktiles, cols] with k = p*72 + kt. Each partition
then reads one contiguous DRAM run per chunk, giving full DMA
efficiency with only a handful of dma_start instructions.
"""

import sys

import numpy as np

_TRN_REPO = "/opt/trn_rl_repo"
if _TRN_REPO not in sys.path:
    sys.path.insert(0, _TRN_REPO)

# Problem shapes (hardcoded per contest rules; kernel.py must be self-contained).
B, N_PREV, D_PREV = 256, 1152, 8
CAPS_N, CAPS_DIM = 10, 16
K = N_PREV * D_PREV            # 9216 contraction length
JK = CAPS_N * CAPS_DIM         # 160 output cols
P = 128                        # SBUF partitions
KT = K // P                    # 72 K-tiles
N_CORES = 8
NB, NJ = 4, 2                  # batch split x capsule-half split
BC = B // NB                   # 64 batch rows per core
JC = JK // NJ                  # 80 cols per core (5 whole capsules)
N_GROUPS = JC // CAPS_DIM      # 5 capsules per core
CHUNKS = 8                     # DMA chunks over the K dimension
TPC = KT // CHUNKS             # 9 K-tiles per chunk
EPS = 1e-7
# softmax(zeros) over 10 classes == fp32(1/10), folded in after the matmul.
ROUTE_C = float(np.float32(1.0) / np.float32(10.0))

_cache: dict = {}


def _build_module():
    from concourse import bacc, mybir
    import concourse.tile as tile

    nc = bacc.Bacc(
        "TRN2",
        debug=False,
        enable_asserts=False,
        target_bir_lowering=False,
        num_devices=N_CORES,
    )
    f32 = mybir.dt.float32
    xt_d = nc.dram_tensor("xt", [P, KT, BC], f32, kind="ExternalInput").ap()
    w_d = nc.dram_tensor("w", [P, KT, JC], f32, kind="ExternalInput").ap()
    out_d = nc.dram_tensor("out", [BC, JC], f32, kind="ExternalOutput").ap()

    with tile.TileContext(nc) as tc:
        _kernel_body(tc, nc, mybir, xt_d, w_d, out_d)

    nc.compile()
    return nc


def _kernel_body(tc, nc, mybir, xt_d, w_d, out_d):
    f32 = mybir.dt.float32
    with (
        tc.tile_pool(name="wp", bufs=CHUNKS) as wp,
        tc.tile_pool(name="xp", bufs=CHUNKS) as xp,
        tc.tile_pool(name="ep", bufs=1) as ep,
        tc.tile_pool(name="pp", bufs=1, space="PSUM") as pp,
    ):
        w_tiles = []
        x_tiles = []
        for c in range(CHUNKS):
            wt = wp.tile([P, TPC, JC], f32)
            nc.sync.dma_start(out=wt[:], in_=w_d[:, c * TPC : (c + 1) * TPC, :])
            xt = xp.tile([P, TPC, BC], f32)
            nc.sync.dma_start(out=xt[:], in_=xt_d[:, c * TPC : (c + 1) * TPC, :])
            w_tiles.append(wt)
            x_tiles.append(xt)

        ps = pp.tile([BC, JC], f32)
        for c in range(CHUNKS):
            for t in range(TPC):
                k = c * TPC + t
                nc.tensor.matmul(
                    ps[:],
                    x_tiles[c][:, t, :],   # lhsT [K=128, M=64]
                    w_tiles[c][:, t, :],   # rhs  [K=128, N=80]
                    start=(k == 0),
                    stop=(k == KT - 1),
                )

        # s = 0.1 * psum  (routing weights are uniform 1/10)
        s = ep.tile([BC, JC], f32)
        nc.scalar.activation(
            s[:], ps[:], mybir.ActivationFunctionType.Copy, bias=0.0, scale=ROUTE_C
        )
        # squash: v = s * [q / ((1+q) * sqrt(q+eps))],  q = sum_k s^2 per capsule
        sq = ep.tile([BC, JC], f32)
        nc.vector.tensor_mul(sq[:], s[:], s[:])
        q = ep.tile([BC, N_GROUPS], f32)
        nc.vector.reduce_sum(
            q[:],
            sq[:].rearrange("p (g k) -> p g k", k=CAPS_DIM),
            axis=mybir.AxisListType.X,
        )
        qe = ep.tile([BC, N_GROUPS], f32)
        nc.vector.tensor_scalar_add(qe[:], q[:], EPS)
        r = ep.tile([BC, N_GROUPS], f32)
        nc.scalar.sqrt(r[:], qe[:])
        onep = ep.tile([BC, N_GROUPS], f32)
        nc.vector.tensor_scalar_add(onep[:], q[:], 1.0)
        den = ep.tile([BC, N_GROUPS], f32)
        nc.vector.tensor_mul(den[:], onep[:], r[:])
        rec = ep.tile([BC, N_GROUPS], f32)
        nc.vector.reciprocal(rec[:], den[:])
        fac = ep.tile([BC, N_GROUPS], f32)
        nc.vector.tensor_mul(fac[:], q[:], rec[:])

        v = ep.tile([BC, JC], f32)
        for g in range(N_GROUPS):
            nc.vector.tensor_scalar_mul(
                v[:, g * CAPS_DIM : (g + 1) * CAPS_DIM],
                s[:, g * CAPS_DIM : (g + 1) * CAPS_DIM],
                fac[:, g : g + 1],
            )
        nc.sync.dma_start(out=out_d[:], in_=v[:])


def _get_module():
    if "nc" not in _cache:
        _cache["nc"] = _build_module()
    return _cache["nc"]


def _make_in_maps(x: np.ndarray, W: np.ndarray) -> list[dict[str, np.ndarray]]:
    # Wmat[(i*8+d), (j*16+k)] = W[0, i, j, k, d]
    wmat = np.ascontiguousarray(W[0].transpose(0, 3, 1, 2)).reshape(K, JK)
    in_maps = []
    for c in range(N_CORES):
        bi, ji = divmod(c, NJ)
        xs = x[bi * BC : (bi + 1) * BC].reshape(BC, K)
        # [K, BC] with k = p*KT + kt, then view as [P, KT, BC]: each
        # partition's chunk is one contiguous DRAM run.
        xtc = np.ascontiguousarray(xs.T).reshape(P, KT, BC)
        wc = np.ascontiguousarray(wmat[:, ji * JC : (ji + 1) * JC]).reshape(P, KT, JC)
        in_maps.append({"xt": xtc, "w": wc})
    return in_maps


def kernel(**inputs) -> np.ndarray:
    from concourse.bass_utils import run_bass_kernel_spmd

    x = np.ascontiguousarray(np.asarray(inputs["x"], dtype=np.float32))
    W = np.ascontiguousarray(np.asarray(inputs["W"], dtype=np.float32))
    assert x.shape == (B, N_PREV, D_PREV), x.shape
    assert W.shape == (1, N_PREV, CAPS_N, CAPS_DIM, D_PREV), W.shape

    in_maps = _make_in_maps(x, W)
    nc = _get_module()
    res = run_bass_kernel_spmd(nc, in_maps, core_ids=list(range(N_CORES)))
    _cache["last_results"] = res

    out = np.empty((B, JK), dtype=np.float32)
    for c, r in enumerate(res.results):
        bi, ji = divmod(c, NJ)
        out[bi * BC : (bi + 1) * BC, ji * JC : (ji + 1) * JC] = r["out"]
    return out.reshape(B, 1, CAPS_N, CAPS_DIM, 1)


# revision 5
# speedup vs baseline: 1.0387x; 1.0387x over previous
"""Trainium2 Bass kernel for the DigitCaps layer (capsule routing).

Math note: in the reference, the routing logits `raw` are never updated
inside the loop (the original model's bug, replicated faithfully), so
c = softmax(zeros) = 1/10 on every iteration and the whole module
collapses to

    v = squash(0.1 * einsum('ijkd,bid->bjk', W[0], x))

i.e. a single matmul  s[b, jk] = x_flat[b, :] @ Wmat[:, jk]  with
x_flat [B, 9216], Wmat [9216, 160] (k-index = (i, d), col = (j, k)),
followed by a per-(b, j) squash over the 16-dim capsule vectors.

Sharding (8 cores, no collectives): batch 4-way x capsule-half 2-way.
Each core computes s_c [64, 80] = xT_c^T @ W_c as 72 PSUM-accumulated
K=128 matmuls, then squashes on-chip. Per-core HBM traffic is
2.36 MB (x quarter) + 2.95 MB (W half) ~= 5.3 MB, the minimum for a
collective-free layout.

Layout trick: the contraction order over k = (i, d) is arbitrary as long
as both operands agree, so host-side we lay out both operands as
[128 partitions, 72 ktiles, cols] with k = p*72 + kt. Each partition
then reads one contiguous DRAM run per chunk, giving full DMA
efficiency with only a handful of dma_start instructions.
"""

import sys

import numpy as np

_TRN_REPO = "/opt/trn_rl_repo"
if _TRN_REPO not in sys.path:
    sys.path.insert(0, _TRN_REPO)

# Problem shapes (hardcoded per contest rules; kernel.py must be self-contained).
B, N_PREV, D_PREV = 256, 1152, 8
CAPS_N, CAPS_DIM = 10, 16
K = N_PREV * D_PREV            # 9216 contraction length
JK = CAPS_N * CAPS_DIM         # 160 output cols
P = 128                        # SBUF partitions
KT = K // P                    # 72 K-tiles
N_CORES = 8
NB, NJ = 4, 2                  # batch split x capsule-half split
BC = B // NB                   # 64 batch rows per core
JC = JK // NJ                  # 80 cols per core (5 whole capsules)
N_GROUPS = JC // CAPS_DIM      # 5 capsules per core
# DMA chunk sizes over the 72 K-tiles: small first chunk for fast pipeline
# spin-up, small last chunk so the PE tail after the final arrival is short.
CHUNK_PLAN = [4, 8, 12, 12, 12, 12, 8, 4]
assert sum(CHUNK_PLAN) == KT
EPS = 1e-7
# softmax(zeros) over 10 classes == fp32(1/10), folded in after the matmul.
ROUTE_C = float(np.float32(1.0) / np.float32(10.0))

_cache: dict = {}


def _build_module():
    from concourse import bacc, mybir
    import concourse.tile as tile

    nc = bacc.Bacc(
        "TRN2",
        debug=False,
        enable_asserts=False,
        target_bir_lowering=False,
        num_devices=N_CORES,
    )
    f32 = mybir.dt.float32
    xt_d = nc.dram_tensor("xt", [P, KT, BC], f32, kind="ExternalInput").ap()
    w_d = nc.dram_tensor("w", [P, KT, JC], f32, kind="ExternalInput").ap()
    out_d = nc.dram_tensor("out", [BC, JC], f32, kind="ExternalOutput").ap()

    with tile.TileContext(nc) as tc:
        _kernel_body(tc, nc, mybir, xt_d, w_d, out_d)

    nc.compile()
    return nc


def _kernel_body(tc, nc, mybir, xt_d, w_d, out_d):
    import concourse.bass as bass

    f32 = mybir.dt.float32
    nchunks = len(CHUNK_PLAN)
    with (
        tc.tile_pool(name="wp", bufs=nchunks) as wp,
        tc.tile_pool(name="xp", bufs=nchunks) as xp,
        tc.tile_pool(name="ep", bufs=1) as ep,
        tc.tile_pool(name="pp", bufs=1, space="PSUM") as pp,
    ):
        # Sqrt activation-table preload: a dummy sqrt issued up front pulls
        # the ~1.3us ACT_TABLE_LOAD out of the epilogue critical path and
        # into the DMA-wait window. eps bias tile for the real sqrt.
        dummy = ep.tile([1, 1], f32)
        nc.vector.memset(dummy[:], 1.0)
        dummy2 = ep.tile([1, 1], f32)
        nc.scalar.sqrt(dummy2[:], dummy[:])
        eps_t = ep.tile([BC, 1], f32)
        nc.vector.memset(eps_t[:], EPS)

        # W chunks issue on the sync (SP) HWDGE queue, x chunks on the
        # scalar (ACT) queue — two sequencers halve the ~650ns/dma issue
        # serialization.
        w_tiles = []
        x_tiles = []
        k0 = 0
        for c, tpc in enumerate(CHUNK_PLAN):
            wt = wp.tile([P, tpc, JC], f32, name=f"wt{c}", tag="wt")
            nc.sync.dma_start(out=wt[:], in_=w_d[:, k0 : k0 + tpc, :])
            xt = xp.tile([P, tpc, BC], f32, name=f"xt{c}", tag="xt")
            nc.scalar.dma_start(out=xt[:], in_=xt_d[:, k0 : k0 + tpc, :])
            w_tiles.append(wt)
            x_tiles.append(xt)
            k0 += tpc

        ps = pp.tile([BC, JC], f32)
        k = 0
        for c, tpc in enumerate(CHUNK_PLAN):
            for t in range(tpc):
                nc.tensor.matmul(
                    ps[:],
                    x_tiles[c][:, t, :],   # lhsT [K=128, M=64]
                    w_tiles[c][:, t, :],   # rhs  [K=128, N=80]
                    start=(k == 0),
                    stop=(k == KT - 1),
                )
                k += 1

        # s = 0.1 * psum  (routing weights are uniform 1/10); DVE reads PSUM
        # directly, avoiding an ACT Copy (and its activation table load).
        s = ep.tile([BC, JC], f32)
        nc.vector.tensor_scalar_mul(s[:], ps[:], ROUTE_C)
        # squash: v = s * [q / ((1+q) * sqrt(q+eps))],  q = sum_k s^2 per capsule
        sq = ep.tile([BC, JC], f32)
        nc.vector.tensor_mul(sq[:], s[:], s[:])
        q = ep.tile([BC, N_GROUPS], f32)
        nc.vector.reduce_sum(
            q[:],
            sq[:].rearrange("p (g k) -> p g k", k=CAPS_DIM),
            axis=mybir.AxisListType.X,
        )
        r = ep.tile([BC, N_GROUPS], f32)
        nc.scalar.activation(
            r[:], q[:], mybir.ActivationFunctionType.Sqrt, bias=eps_t[:]
        )
        onep = ep.tile([BC, N_GROUPS], f32)
        nc.vector.tensor_scalar_add(onep[:], q[:], 1.0)
        den = ep.tile([BC, N_GROUPS], f32)
        nc.vector.tensor_mul(den[:], onep[:], r[:])
        rec = ep.tile([BC, N_GROUPS], f32)
        nc.vector.reciprocal(rec[:], den[:])
        fac = ep.tile([BC, N_GROUPS], f32)
        nc.vector.tensor_mul(fac[:], q[:], rec[:])

        v = ep.tile([BC, JC], f32)
        fac_bcast = bass.AP(
            tensor=fac[:].tensor,
            offset=fac[:].offset,
            ap=[*fac[:].ap, [0, CAPS_DIM]],
        )
        nc.vector.tensor_mul(
            v[:].rearrange("p (g k) -> p g k", k=CAPS_DIM),
            s[:].rearrange("p (g k) -> p g k", k=CAPS_DIM),
            fac_bcast,
        )
        nc.sync.dma_start(out=out_d[:], in_=v[:])


def _get_module():
    if "nc" not in _cache:
        _cache["nc"] = _build_module()
    return _cache["nc"]


def _make_in_maps(x: np.ndarray, W: np.ndarray) -> list[dict[str, np.ndarray]]:
    # Wmat[(i*8+d), (j*16+k)] = W[0, i, j, k, d]
    wmat = np.ascontiguousarray(W[0].transpose(0, 3, 1, 2)).reshape(K, JK)
    in_maps = []
    for c in range(N_CORES):
        bi, ji = divmod(c, NJ)
        xs = x[bi * BC : (bi + 1) * BC].reshape(BC, K)
        # [K, BC] with k = p*KT + kt, then view as [P, KT, BC]: each
        # partition's chunk is one contiguous DRAM run.
        xtc = np.ascontiguousarray(xs.T).reshape(P, KT, BC)
        wc = np.ascontiguousarray(wmat[:, ji * JC : (ji + 1) * JC]).reshape(P, KT, JC)
        in_maps.append({"xt": xtc, "w": wc})
    return in_maps


def kernel(**inputs) -> np.ndarray:
    from concourse.bass_utils import run_bass_kernel_spmd

    x = np.ascontiguousarray(np.asarray(inputs["x"], dtype=np.float32))
    W = np.ascontiguousarray(np.asarray(inputs["W"], dtype=np.float32))
    assert x.shape == (B, N_PREV, D_PREV), x.shape
    assert W.shape == (1, N_PREV, CAPS_N, CAPS_DIM, D_PREV), W.shape

    in_maps = _make_in_maps(x, W)
    nc = _get_module()
    res = run_bass_kernel_spmd(nc, in_maps, core_ids=list(range(N_CORES)))
    _cache["last_results"] = res

    out = np.empty((B, JK), dtype=np.float32)
    for c, r in enumerate(res.results):
        bi, ji = divmod(c, NJ)
        out[bi * BC : (bi + 1) * BC, ji * JC : (ji + 1) * JC] = r["out"]
    return out.reshape(B, 1, CAPS_N, CAPS_DIM, 1)
